# revision 31
# baseline (speedup 1.0000x reference)
"""AdaptiveConv2DMod Trainium2 kernel (v4).

Per-sample modulated 3x3 conv (StyleGAN2-style) on 8 NeuronCores,
data-parallel over batch (1 sample per core, no collectives).

HW profile history (NTFF exec time, core 0, 8 cores running):
  v2 (f32 HBM, staged DVE convert)                  ~278 us
  v3 (bf16 HBM boundary, direct-to-tape loads)      ~212 us
  v4 (de-gpsimd'd prep, SBLK=16, bf16 weight bank)  ~209 us

Key facts this design is built around (measured on trn2.8x1):
  - The 16 DMA engines deliver ~15-19 GB/s each (~250 GB/s/core
    aggregate under 8-core load) REGARDLESS of descriptor size, so HBM
    bytes are the wall: ship bf16 across HBM in both directions.  The
    host casts fmap to bf16 (identical RNE rounding to the on-chip DVE
    convert it replaces) and casts the bf16 output back to f32
    (+~0.1% rms on a 2e-2 budget).  33.6 MB total -> ~134 us of DMA,
    fully hidden behind the PE.
  - PE streaming is then the bottleneck: 768 matmuls x 512 columns
    (bf16, 1 col/cycle @2.4GHz) = 164 us floor, ~184 us active with
    DVFS/throttle; the kernel runs at ~88% PE occupancy.
  - gpsimd(Q7) tensor ops cost ~10 us of launch latency EACH on HW
    (cost model says 95 ns) — keep them off the critical path; the
    attn broadcast runs as a K=1 PE matmul instead.
  - Strip-major HBM layouts (host re/unstripes) keep every DMA call a
    single 128-partition descriptor batch:
      fmap4[g, c, t, x] = fmap[c, 4t+g, x]            (bf16 in)
      out5[j, oc, u, x] = out row 4u+1+j; (3,:,T-1) = row 0 (bf16 out)

Per-core layout (offset-bank scheme):
  - Tape X: image row r at partition strip r%4 (32 channels each),
    slot r//4, width padded to W+4 with zero columns.  bf16, whole
    image resident (nslot=T=128, 16.5 MB SBUF), no wrap; input DMAs
    land straight in the tape (no staging, no convert pass).
  - Output bank t covers rows 4t+1..4t+4; its 6 input rows (4t..4t+5)
    split 4+2 across slots t, t+1 -> 6 matmuls per bank, all M=128:
      A1(sigma): K=128 over slot t     (9 nonzero 32x32 weight blocks)
      A2(sigma): K=64  over slot t+1 strips 0-1 (3 nonzero blocks),
    sigma = kx in {0,1,2} as a +sigma column offset in the rhs AP.
    (6/bank is provably minimal for this tape: the 12 (ro,ri) band
    pairs per sigma need >=2 matmuls because ri spans 6 rows > K.)
  - Edge banks: bank -1 (row 0 via its A2 only) and bank 127 (rows
    509..511 via its A1 only, computed early at group NG-2).
  - Weights (softmax-mixed + modulated + demodulated, built on device
    in f32 from the bf16 bank) are transposed into WA1 [128, 3*128] /
    WA2 [64, 3*128] by PE transposes into PSUM + DVE/ACT evacuations.
  - Output: ACT evacuates PSUM->bf16 otiles (8 banks each); fused
    128-partition SWDGE DMAs per half-block (separate semaphore pool
    from the HWDGE input loads so prefetch never gates on output);
    the last banks stream out per-bank on the ACT HWDGE queue.
"""

import sys

import numpy as np

try:
    import concourse.bass as bass  # noqa: F401
except ImportError:
    sys.path.insert(0, "/opt/trn_rl_repo")

import concourse.bass as bass
import concourse.tile as tile
from concourse import bacc, mybir
from concourse.bass_utils import run_bass_kernel_spmd

F32 = mybir.dt.float32
BF16 = mybir.dt.bfloat16

C = 32          # in/out channels
NK = 4          # kernel bank size
EPS = 1e-8


def build_graph(H=512, W=512, nslot=128, ablate="", timing=False, repeat=1):
    """Build the per-core Bass graph. Returns compiled Bacc.

    ablate: comma-set of stages to skip ("mm", "evac", "odma") for
    TimelineSim bottleneck analysis only.
    timing: write the image to an Internal DRAM scratch and expose only a
    tiny external output, so repeated executions can be queued back-to-back
    without device-memory pressure (wall-clock delta timing).
    """
    skip = set(ablate.split(",")) if ablate else set()
    T = H // 4                      # tape slots / banks per image
    nslot = min(nslot, T)
    Wp = W + 4                      # padded width

    nc = bacc.Bacc("TRN2", target_bir_lowering=False, debug=False)

    fmap = nc.dram_tensor("fmap", [4, C, T, W], BF16, kind="ExternalInput")
    mod = nc.dram_tensor("mod", [1, C], F32, kind="ExternalInput")
    kmod = nc.dram_tensor("kernel_mod", [1, NK], F32, kind="ExternalInput")
    wbank = nc.dram_tensor("weights", [NK, C, C, 3, 3], BF16, kind="ExternalInput")
    ident = nc.inline_tensor(np.eye(C, dtype=np.float32), name="ident32")
    if timing:
        out = nc.dram_tensor("out", [1, NK], BF16, kind="ExternalOutput")
        oscr = nc.dram_tensor("oscr", [4, C, T, W], BF16, kind="Internal")
        osink = oscr
    else:
        out = nc.dram_tensor("out", [4, C, T, W], BF16, kind="ExternalOutput")
        osink = out

    SBLK = min(16, T)               # slots per input-DMA block
    NBLK = T // SBLK
    fm4 = fmap.ap().rearrange("g i t x -> (g i) t x")

    with tile.TileContext(nc) as tc:
        with (
            tc.tile_pool(name="xpool", bufs=1) as xpool,
            tc.tile_pool(name="wpool", bufs=1) as wpool,
            tc.tile_pool(name="cpool", bufs=1) as cpool,
        ):
            X = xpool.tile([128, nslot * Wp], BF16)
            WA1 = wpool.tile([128, 3 * 128], BF16)    # sigma-major A1 lhsT
            WA2 = wpool.tile([64, 3 * 128], BF16)     # sigma-major A2 lhsT
            id32 = cpool.tile([C, C], F32)
            ones1 = cpool.tile([1, C], F32)
            m1B = cpool.tile([128, 1], F32)           # (1+mod[ci]) tiled x4
            e0 = cpool.tile([C, W], BF16)             # row 0 staging
            e1 = cpool.tile([96, W], BF16)            # rows H-3..H-1 staging
            s4 = cpool.tile([1, NK], BF16)            # timing-mode sink

            nc.scalar.dma_start(id32[:, :], ident.ap())
            nc.vector.memset(ones1[:, :], 1.0)
            # zero weight tiles: only valid blocks are copied in
            nc.vector.memset(WA1[:, :], 0.0)
            nc.vector.memset(WA2[:, :], 0.0)

            # zero the pad columns of every slot (input DMA writes only
            # columns 2..2+W, so the pads stay zero)
            xv = X[:, :].rearrange("p (s q) -> p s q", q=Wp)
            nc.vector.memset(xv[:, :, 0:2], 0.0)
            nc.vector.memset(xv[:, :, Wp - 2:Wp], 0.0)

            # ---------------- weight preparation ----------------
            with (
                tc.tile_pool(name="prep", bufs=2) as prep,
                tc.tile_pool(name="prep_ps", bufs=2, space="PSUM") as prep_ps,
            ):
                # softmax(kernel_mod) -> attn [1, NK].  No max-subtraction:
                # kernel_mod ~ N(0,1), exp stays far from f32 overflow.
                km = prep.tile([1, NK], F32)
                nc.scalar.dma_start(km[:, :], kmod.ap())
                ex = prep.tile([1, NK], F32)
                nc.scalar.activation(
                    ex[:, :], km[:, :], mybir.ActivationFunctionType.Exp,
                )
                sm = prep.tile([1, 1], F32)
                nc.vector.reduce_sum(sm[:, :], ex[:, :], axis=mybir.AxisListType.X)
                rs = prep.tile([1, 1], F32)
                nc.vector.reciprocal(rs[:, :], sm[:, :])
                attn = prep.tile([1, NK], F32)
                nc.vector.tensor_scalar_mul(attn[:, :], ex[:, :], rs[:, 0:1])

                # broadcast attn to all C partitions via a K=1 PE matmul
                # (gpsimd partition_broadcast = Q7 launch = ~10us on HW)
                attnB = prep.tile([C, NK], F32)
                ps_b = prep_ps.tile([C, NK], F32, tag="psb")
                nc.tensor.matmul(
                    ps_b[:, :], ones1[0:1, :], attn[:, :],
                    start=True, stop=True)
                nc.vector.tensor_copy(attnB[:, :], ps_b[:, :])

                # P[o, n*288 + i*9 + tap] = weights[n, o, i, ky, kx]
                # (bf16: halves the 1.15MB load on the startup critical
                # path; the f32 mix/demod math sees bf16-rounded weights,
                # identical to what lands in WA1/WA2 anyway)
                P = prep.tile([C, NK * 288], BF16)
                nc.scalar.dma_start(
                    P[:, :], wbank.ap().rearrange("n o i ky kx -> o n (i ky kx)")
                )

                # mix[o, i*9+tap] = sum_n attn[n] * P[o, n, ...]
                # (DVE mul + ACT scaled-copy pairs, tree-added on DVE; no
                # gpsimd — each Q7 launch costs ~10us of real latency)
                mix = prep.tile([C, 288], F32, tag="mix")
                t0 = prep.tile([C, 288], F32, tag="t0")
                t1 = prep.tile([C, 288], F32, tag="t1")
                t2 = prep.tile([C, 288], F32, tag="t2")
                nc.vector.tensor_scalar_mul(t0[:, :], P[:, 0:288], attnB[:, 0:1])
                nc.scalar.activation(
                    t1[:, :], P[:, 288:576],
                    mybir.ActivationFunctionType.Copy, scale=attnB[:, 1:2])
                nc.vector.tensor_scalar_mul(
                    t2[:, :], P[:, 576:864], attnB[:, 2:3])
                nc.scalar.activation(
                    mix[:, :], P[:, 864:1152],
                    mybir.ActivationFunctionType.Copy, scale=attnB[:, 3:4])
                nc.vector.tensor_add(t0[:, :], t0[:, :], t1[:, :])
                nc.vector.tensor_add(t2[:, :], t2[:, :], mix[:, :])
                nc.vector.tensor_add(mix[:, :], t0[:, :], t2[:, :])

                # mvec[i, 1] = mod + 1 ;  m2 = mvec^2 ; m1B = mvec tiled x4
                mv = prep.tile([C, 1], F32, tag="mv")
                nc.scalar.dma_start(mv[:, :], mod.ap().rearrange("a i -> i a"))
                m1 = prep.tile([C, 1], F32, tag="m1")
                nc.scalar.add(m1[:, :], mv[:, :], 1.0)
                m2 = prep.tile([C, 1], F32, tag="m2")
                nc.vector.tensor_mul(m2[:, :], m1[:, :], m1[:, :])
                for g in range(4):
                    nc.scalar.dma_start(
                        m1B[32 * g:32 * g + 32, :], mod.ap().rearrange("a i -> i a")
                    )
                nc.scalar.add(m1B[:, :], m1B[:, :], 1.0)

                # demodulation: inv[o] = rsqrt(sum_{i,tap} (mix * m1[i])^2)
                sq = prep.tile([C, 288], F32, tag="tmp")
                nc.vector.tensor_mul(sq[:, :], mix[:, :], mix[:, :])
                s_oi = prep.tile([C, C], F32, tag="soi")
                nc.vector.reduce_sum(
                    s_oi[:, :],
                    sq[:, :].rearrange("p (i t) -> p i t", t=9),
                    axis=mybir.AxisListType.X,
                )
                ps_a = prep_ps.tile([C, C], F32, tag="psa")
                nc.tensor.transpose(ps_a[:, :], s_oi[:, :], id32[:, :])
                sT = prep.tile([C, C], F32, tag="soi")
                nc.vector.tensor_copy(sT[:, :], ps_a[:, :])

                ps_n = prep_ps.tile([1, C], F32, tag="psa")
                nc.tensor.matmul(
                    ps_n[:, :], m2[:, :], sT[:, :], start=True, stop=True
                )
                ns = prep.tile([1, C], F32, tag="ns")
                nc.vector.tensor_scalar_max(ns[:, :], ps_n[:, :], EPS)
                sqn = prep.tile([1, C], F32, tag="sqn")
                nc.scalar.sqrt(sqn[:, :], ns[:, :])
                inv = prep.tile([1, C], F32, tag="inv")
                nc.vector.reciprocal(inv[:, :], sqn[:, :])

                ps_i = prep_ps.tile([C, 1], F32, tag="psa")
                nc.tensor.transpose(ps_i[:, :], inv[:, :], ones1[:, 0:1])
                invT = prep.tile([C, 1], F32, tag="invT")
                nc.vector.tensor_copy(invT[:, :], ps_i[:, :])

                # Build gathered pre-transpose layouts in SBUF (DVE, within-
                # partition column shuffles), then transpose each out-column
                # block [32, Kblk] -> [Kblk, 32] so every matmul PSUM output
                # sits at a legal PE tile position.
                # mix4 view: [o, kx, ky, ci]
                mix4 = mix[:, :].rearrange(
                    "o (i ky kx) -> o kx ky i", ky=3, kx=3)
                # wtAB block (sg, c) at col (4*sg+c)*128:
                #   cols 32*(c+ky)+ci = mix[o, ci, ky, sg] * inv[o]
                wtAB = prep.tile([C, 12 * 128], F32, name="wtAB")
                nc.vector.memset(wtAB[:, :], 0.0)
                for sg in range(3):
                    for c in range(4):
                        nky = min(3, 4 - c)
                        base = (4 * sg + c) * 128 + 32 * c
                        nc.vector.tensor_scalar_mul(
                            wtAB[:, base:base + 32 * nky].rearrange(
                                "o (ky i) -> o ky i", i=C),
                            mix4[:, sg, 0:nky, :],
                            invT[:, 0:1],
                        )
                # wtAB2 block (sg, c in {2,3}) at col 128*sg + 64*(c-2):
                #   cols 32*p'+ci = w[ky=p'+4-c] (c=2: p'=0 ky=2;
                #   c=3: p'=ky-1, ky in {1,2})
                wtAB2 = prep.tile([C, 3 * 128], F32, name="wtAB2")
                nc.vector.memset(wtAB2[:, :], 0.0)
                for sg in range(3):
                    nc.vector.tensor_scalar_mul(
                        wtAB2[:, 128 * sg:128 * sg + 32],
                        mix4[:, sg, 2, :],
                        invT[:, 0:1],
                    )
                    nc.vector.tensor_scalar_mul(
                        wtAB2[:, 128 * sg + 64:128 * sg + 128].rearrange(
                            "o (ky i) -> o ky i", i=C),
                        mix4[:, sg, 1:3, :],
                        invT[:, 0:1],
                    )
                # PE transposes: wide blocks -> PSUM at partition 0
                pw1 = prep_ps.tile([128, 3 * 128], F32, tag="pw1")
                tps = [(sg, c) for sg in range(3) for c in range(4)]
                for i, (sg, c) in enumerate(tps):
                    nc.tensor.matmul(
                        pw1[:, 128 * sg + 32 * c:128 * sg + 32 * c + 32],
                        wtAB[:, (4 * sg + c) * 128:(4 * sg + c) * 128 + 128],
                        id32[:, :],
                        is_transpose=True,
                        start=(i == 0), stop=(i == len(tps) - 1),
                    )
                pw2 = prep_ps.tile([64, 3 * 128], F32, tag="pw2")
                tps2 = [(sg, c) for sg in range(3) for c in (2, 3)]
                for i, (sg, c) in enumerate(tps2):
                    nc.tensor.matmul(
                        pw2[:, 128 * sg + 32 * c:128 * sg + 32 * c + 32],
                        wtAB2[:, 128 * sg + 64 * (c - 2):
                              128 * sg + 64 * (c - 2) + 64],
                        id32[:, :],
                        is_transpose=True,
                        start=(i == 0), stop=(i == len(tps2) - 1),
                    )
                # evacuate to bf16 weight tiles, scaled by (1+mod[ci]);
                # WA1 on DVE, WA2 on ACT (parallel streams)
                for sg in range(3):
                    nc.vector.tensor_scalar_mul(
                        WA1[:, 128 * sg:128 * sg + 128],
                        pw1[:, 128 * sg:128 * sg + 128],
                        m1B[:, 0:1],
                    )
                    nc.vector.tensor_scalar_mul(
                        WA2[0:64, 128 * sg + 64:128 * sg + 128],
                        pw2[0:64, 128 * sg + 64:128 * sg + 128],
                        m1B[0:64, 0:1],
                    )

            # ---------------- main conv loop ----------------
            with (
                tc.tile_pool(name="cps", bufs=8, space="PSUM") as cps,
                tc.tile_pool(name="opool", bufs=6) as opool,
            ):
                def load_slots(b, lo, nsl):
                    # DMA block b slots [lo, lo+nsl) straight into the tape
                    # (one 128-partition call; per-partition src contiguous)
                    p0 = (b * SBLK + lo) % nslot
                    s0 = b * SBLK + lo
                    nc.sync.dma_start(
                        xv[:, p0:p0 + nsl, 2:2 + W],
                        fm4[:, s0:s0 + nsl, :],
                    )

                def mm_bank(pts_t, t, sigma, which, first, last):
                    # one stream for bank t (out rows 4t+1..4t+4)
                    if which == 1:          # A1: K=128 over slot t
                        st = t % nslot
                        lhsT = WA1[:, 128 * sigma:128 * sigma + 128]
                        rhs = X[0:128, st * Wp + 1 + sigma:
                                st * Wp + 1 + sigma + W]
                        o = pts_t[:, :]
                    else:                   # A2: K=64 over slot t+1 strips 0-1
                        s1 = (t + 1) % nslot
                        lhsT = WA2[0:64, 128 * sigma:128 * sigma + 128]
                        rhs = X[0:64, s1 * Wp + 1 + sigma:
                                s1 * Wp + 1 + sigma + W]
                        o = pts_t[:, :]
                    nc.tensor.matmul(
                        o, lhsT, rhs, start=first, stop=last,
                        skip_group_check=True,
                    )

                for _rep in range(repeat):
                    # stream the whole image in, in arrival order; the first
                    # chunks are small so bank group 0 can start early
                    load_slots(0, 0, 4)
                    load_slots(0, 4, 4)
                    if SBLK > 8:
                        load_slots(0, 8, SBLK - 8)
                    for b in range(1, NBLK):
                        load_slots(b, 0, SBLK)

                    otiles = {}
                    OB = 8                  # banks per output half-block
                    NG = T // 4             # 32 groups of 4 banks
                    for k in range(NG):
                        banks = [t for t in range(4 * k - 1, 4 * k + 3)
                                 if t < T - 1]
                        if k % 2 == 0:
                            h = k // 2
                            otiles[h] = opool.tile(
                                [128, OB * W], BF16, name=f"ot{h}", tag="ot")

                        if "mm" not in skip:
                            pts = {t: cps.tile([128, W], F32,
                                               name=f"pt{t}", tag="pt")
                                   for t in banks}
                            # order: A1s0 (start, full bank) -> A2s (interior
                            # accumulates) -> A1s1 -> A1s2 (stop, full bank).
                            # The closing matmul must span all 128 partitions
                            # so every started psum group is closed.
                            for t in banks:
                                if t >= 0:
                                    mm_bank(pts[t], t, 0, 1,
                                            first=True, last=False)
                            for sigma in range(3):
                                for t in banks:
                                    mm_bank(pts[t], t, sigma, 2,
                                            first=(t < 0 and sigma == 0),
                                            last=(t < 0 and sigma == 2))
                            for sigma in (1, 2):
                                for t in banks:
                                    if t >= 0:
                                        mm_bank(pts[t], t, sigma, 1,
                                                first=False,
                                                last=(sigma == 2))

                        # evacuations (ACT) + per-block output DMA (ACT HWDGE)
                        for t in banks:
                            if "evac" in skip:
                                continue
                            if t < 0:
                                nc.scalar.copy(e0[:, :], pts[t][96:128, :])
                                if "odma" not in skip:
                                    nc.gpsimd.dma_start(
                                        osink.ap()[3, :, T - 1, :],
                                        e0[:, :],
                                    )
                            else:
                                hw_ = (t % OB) * W
                                nc.scalar.copy(
                                    otiles[t // OB][:, hw_:hw_ + W],
                                    pts[t][:, :])
                        if k % 2 == 0 and k > 0 and "odma" not in skip \
                                and "evac" not in skip:
                            hdone = k // 2 - 1
                            _emit_half_dma(nc, osink, otiles[hdone], hdone,
                                           0, OB, W)
                            otiles.pop(hdone, None)
                        if k == NG - 2 and "mm" not in skip:
                            # tail bank T-1 (rows H-3..H-1) early: its slot
                            # (T-1) loaded long ago and nothing depends on it
                            ptl = cps.tile([128, W], F32, name="ptl", tag="pt")
                            for sigma in range(3):
                                mm_bank(ptl, T - 1, sigma, 1,
                                        first=(sigma == 0), last=(sigma == 2))
                            if "evac" not in skip:
                                nc.scalar.copy(e1[:, :], ptl[0:96, :])
                                if "odma" not in skip:
                                    nc.scalar.dma_start(
                                        osink.ap().rearrange(
                                            "c o u x -> (c o) u x")[
                                            0:96, T - 1, :],
                                        e1[:, :],
                                    )
                        if k == NG - 1 and "odma" not in skip \
                                and "evac" not in skip:
                            # first chunk of the last half-block (banks
                            # 120..123 are evacuated by group NG-2)
                            _emit_half_dma(nc, osink, otiles[T // OB - 1],
                                           T // OB - 1, 0, 4, W)

                    # tail: last 3 banks stream out one by one as evacuated
                    if "evac" not in skip and "odma" not in skip:
                        hlast = T // OB - 1
                        for j in range(4, OB - 1):
                            u0 = OB * hlast + j
                            nc.scalar.dma_start(
                                osink.ap().rearrange(
                                    "c o u x -> (c o) (u x)")[
                                    :, u0 * W:(u0 + 1) * W],
                                otiles[hlast][:, j * W:(j + 1) * W],
                            )
                        otiles.pop(hlast, None)

                if timing:
                    nc.sync.dma_start(s4[:, :], osink.ap()[0, 0:1, 0, 0:NK])
                    nc.sync.dma_start(out.ap(), s4[:, :])

    nc.compile()
    return nc


OB8 = 8


def _emit_half_dma(nc, osink, ot, h, lo, nslots, W):
    """Output DMA for half-block h, banks u = 8h+lo .. 8h+lo+nslots-1.

    One 128-partition SWDGE call on the Pool queue (separate semaphore
    pool from the HWDGE input loads, so input prefetch never gates on
    output completion).  osink is strip-major [4, C, T, W] bf16; per
    (strip, channel) partition the dst run over (u, x) is contiguous.
    """
    u0 = OB8 * h + lo
    ov = osink.ap().rearrange("c o u x -> (c o) (u x)")
    nc.gpsimd.dma_start(
        ov[:, u0 * W:(u0 + nslots) * W],
        ot[:, lo * W:(lo + nslots) * W],
    )


_CACHE = {}


def _get_graph(H, W):
    key = (H, W)
    if key not in _CACHE:
        _CACHE[key] = build_graph(H, W)
    return _CACHE[key]


def _bf16():
    import ml_dtypes
    return ml_dtypes.bfloat16


def prep_weights(w):
    """weight bank -> contiguous bf16 (matches on-chip WA precision)."""
    return np.ascontiguousarray(
        np.asarray(w, dtype=np.float32).astype(_bf16()))


def restripe_fmap(fm):
    """[C, H, W] f32 -> [4, C, H//4, W] bf16 with out[g, c, t] = fm[c, 4t+g].

    The bf16 cast (round-to-nearest-even) matches the on-chip DVE convert
    it replaces, so kernel numerics are unchanged.
    """
    C_, H_, W_ = fm.shape
    return np.ascontiguousarray(
        np.asarray(fm, dtype=np.float32).astype(_bf16())
        .reshape(C_, H_ // 4, 4, W_).transpose(2, 0, 1, 3)
    )


def unstripe_out(o5, H):
    """[4, C, T, W] bf16 strip-major kernel output -> [C, H, W] f32."""
    _, C_, T_, W_ = o5.shape
    out = np.empty((C_, H, W_), np.float32)
    out[:, 1::4] = o5[0]
    out[:, 2::4] = o5[1]
    out[:, 3::4] = o5[2]
    out[:, 4::4] = o5[3][:, :T_ - 1]
    out[:, 0] = o5[3][:, T_ - 1]
    return out


def kernel(fmap, mod, kernel_mod, weights):
    B, Ci, H, Wd = fmap.shape
    nc = _get_graph(H, Wd)
    in_maps = [
        {
            "fmap": restripe_fmap(fmap[b]),
            "mod": np.ascontiguousarray(mod[b:b + 1], dtype=np.float32),
            "kernel_mod": np.ascontiguousarray(kernel_mod[b:b + 1], dtype=np.float32),
            "weights": prep_weights(weights),
        }
        for b in range(B)
    ]
    res = run_bass_kernel_spmd(nc, in_maps, core_ids=list(range(B)))
    return np.stack(
        [unstripe_out(res.results[b]["out"], H) for b in range(B)], axis=0
    )


# revision 33
# speedup vs baseline: 1.0437x; 1.0437x over previous
"""AdaptiveConv2DMod Trainium2 kernel (v4).

Per-sample modulated 3x3 conv (StyleGAN2-style) on 8 NeuronCores,
data-parallel over batch (1 sample per core, no collectives).

HW profile history (NTFF exec time, core 0, 8 cores running):
  v2 (f32 HBM, staged DVE convert)                  ~278 us
  v3 (bf16 HBM boundary, direct-to-tape loads)      ~212 us
  v4 (de-gpsimd'd prep, SBLK=16, bf16 weight bank)  ~209 us

Key facts this design is built around (measured on trn2.8x1):
  - The 16 DMA engines deliver ~15-19 GB/s each (~250 GB/s/core
    aggregate under 8-core load) REGARDLESS of descriptor size, so HBM
    bytes are the wall: ship bf16 across HBM in both directions.  The
    host casts fmap to bf16 (identical RNE rounding to the on-chip DVE
    convert it replaces) and casts the bf16 output back to f32
    (+~0.1% rms on a 2e-2 budget).  33.6 MB total -> ~134 us of DMA,
    fully hidden behind the PE.
  - PE streaming is then the bottleneck: 768 matmuls x 512 columns
    (bf16, 1 col/cycle @2.4GHz) = 164 us floor, ~184 us active with
    DVFS/throttle; the kernel runs at ~88% PE occupancy.
  - gpsimd(Q7) tensor ops cost ~10 us of launch latency EACH on HW
    (cost model says 95 ns) — keep them off the critical path; the
    attn broadcast runs as a K=1 PE matmul instead.
  - Strip-major HBM layouts (host re/unstripes) keep every DMA call a
    single 128-partition descriptor batch:
      fmap4[g, c, t, x] = fmap[c, 4t+g, x]            (bf16 in)
      out5[j, oc, u, x] = out row 4u+1+j; (3,:,T-1) = row 0 (bf16 out)

Per-core layout (offset-bank scheme):
  - Tape X: image row r at partition strip r%4 (32 channels each),
    slot r//4, width padded to W+4 with zero columns.  bf16, whole
    image resident (nslot=T=128, 16.5 MB SBUF), no wrap; input DMAs
    land straight in the tape (no staging, no convert pass).
  - Output bank t covers rows 4t+1..4t+4; its 6 input rows (4t..4t+5)
    split 4+2 across slots t, t+1 -> 6 matmuls per bank, all M=128:
      A1(sigma): K=128 over slot t     (9 nonzero 32x32 weight blocks)
      A2(sigma): K=64  over slot t+1 strips 0-1 (3 nonzero blocks),
    sigma = kx in {0,1,2} as a +sigma column offset in the rhs AP.
    (6/bank is provably minimal for this tape: the 12 (ro,ri) band
    pairs per sigma need >=2 matmuls because ri spans 6 rows > K.)
  - Edge banks: bank -1 (row 0 via its A2 only) and bank 127 (rows
    509..511 via its A1 only, computed early at group NG-2).
  - Weights (softmax-mixed + modulated + demodulated, built on device
    in f32 from the bf16 bank) are transposed into WA1 [128, 3*128] /
    WA2 [64, 3*128] by PE transposes into PSUM + DVE/ACT evacuations.
  - Output: ACT evacuates PSUM->bf16 otiles (8 banks each); fused
    128-partition SWDGE DMAs per half-block (separate semaphore pool
    from the HWDGE input loads so prefetch never gates on output);
    the last banks stream out per-bank on the ACT HWDGE queue.
"""

import sys

import numpy as np

try:
    import concourse.bass as bass  # noqa: F401
except ImportError:
    sys.path.insert(0, "/opt/trn_rl_repo")

import concourse.bass as bass
import concourse.tile as tile
from concourse import bacc, mybir
from concourse.bass_utils import run_bass_kernel_spmd

F32 = mybir.dt.float32
BF16 = mybir.dt.bfloat16

C = 32          # in/out channels
NK = 4          # kernel bank size
EPS = 1e-8


def build_graph(H=512, W=512, nslot=128, ablate="", timing=False, repeat=1):
    """Build the per-core Bass graph. Returns compiled Bacc.

    ablate: comma-set of stages to skip ("mm", "evac", "odma") for
    TimelineSim bottleneck analysis only.
    timing: write the image to an Internal DRAM scratch and expose only a
    tiny external output, so repeated executions can be queued back-to-back
    without device-memory pressure (wall-clock delta timing).
    """
    skip = set(ablate.split(",")) if ablate else set()
    T = H // 4                      # tape slots / banks per image
    nslot = min(nslot, T)
    Wp = W + 4                      # padded width

    nc = bacc.Bacc("TRN2", target_bir_lowering=False, debug=False)

    fmap = nc.dram_tensor("fmap", [4, C, T, W], BF16, kind="ExternalInput")
    mod = nc.dram_tensor("mod", [1, C], F32, kind="ExternalInput")
    kmod = nc.dram_tensor("kernel_mod", [1, NK], F32, kind="ExternalInput")
    wbank = nc.dram_tensor("weights", [NK, C, C, 3, 3], BF16, kind="ExternalInput")
    ident = nc.inline_tensor(np.eye(C, dtype=np.float32), name="ident32")
    if timing:
        out = nc.dram_tensor("out", [1, NK], BF16, kind="ExternalOutput")
        oscr = nc.dram_tensor("oscr", [4, C, T, W], BF16, kind="Internal")
        osink = oscr
    else:
        out = nc.dram_tensor("out", [4, C, T, W], BF16, kind="ExternalOutput")
        osink = out

    SBLK = min(16, T)               # slots per input-DMA block
    NBLK = T // SBLK
    fm4 = fmap.ap().rearrange("g i t x -> (g i) t x")

    with tile.TileContext(nc) as tc:
        with (
            tc.tile_pool(name="xpool", bufs=1) as xpool,
            tc.tile_pool(name="wpool", bufs=1) as wpool,
            tc.tile_pool(name="cpool", bufs=1) as cpool,
        ):
            X = xpool.tile([128, nslot * Wp], BF16)
            WA1 = wpool.tile([128, 3 * 128], BF16)    # sigma-major A1 lhsT
            WA2 = wpool.tile([64, 3 * 128], BF16)     # sigma-major A2 lhsT
            id32 = cpool.tile([C, C], F32)
            ones1 = cpool.tile([1, C], F32)
            m1B = cpool.tile([128, 1], F32)           # (1+mod[ci]) tiled x4
            e0 = cpool.tile([C, W], BF16)             # row 0 staging
            e1 = cpool.tile([96, W], BF16)            # rows H-3..H-1 staging
            s4 = cpool.tile([1, NK], BF16)            # timing-mode sink

            nc.sync.dma_start(id32[:, :], ident.ap())
            nc.vector.memset(ones1[:, :], 1.0)
            # zero weight tiles: only valid blocks are copied in
            nc.vector.memset(WA1[:, :], 0.0)
            nc.vector.memset(WA2[:, :], 0.0)

            # zero the pad columns of every slot (input DMA writes only
            # columns 2..2+W, so the pads stay zero)
            xv = X[:, :].rearrange("p (s q) -> p s q", q=Wp)
            nc.vector.memset(xv[:, :, 0:2], 0.0)
            nc.vector.memset(xv[:, :, Wp - 2:Wp], 0.0)

            # ---------------- weight preparation ----------------
            with (
                tc.tile_pool(name="prep", bufs=2) as prep,
                tc.tile_pool(name="prep_ps", bufs=2, space="PSUM") as prep_ps,
            ):
                # softmax(kernel_mod) -> attn [1, NK].  No max-subtraction:
                # kernel_mod ~ N(0,1), exp stays far from f32 overflow.
                km = prep.tile([1, NK], F32)
                nc.sync.dma_start(km[:, :], kmod.ap())
                ex = prep.tile([1, NK], F32)
                nc.scalar.activation(
                    ex[:, :], km[:, :], mybir.ActivationFunctionType.Exp,
                )
                sm = prep.tile([1, 1], F32)
                nc.vector.reduce_sum(sm[:, :], ex[:, :], axis=mybir.AxisListType.X)
                rs = prep.tile([1, 1], F32)
                nc.vector.reciprocal(rs[:, :], sm[:, :])
                attn = prep.tile([1, NK], F32)
                nc.vector.tensor_scalar_mul(attn[:, :], ex[:, :], rs[:, 0:1])

                # broadcast attn to all C partitions via a K=1 PE matmul
                # (gpsimd partition_broadcast = Q7 launch = ~10us on HW)
                attnB = prep.tile([C, NK], F32)
                ps_b = prep_ps.tile([C, NK], F32, tag="psb")
                nc.tensor.matmul(
                    ps_b[:, :], ones1[0:1, :], attn[:, :],
                    start=True, stop=True)
                nc.vector.tensor_copy(attnB[:, :], ps_b[:, :])

                # P[o, n*288 + i*9 + tap] = weights[n, o, i, ky, kx]
                # (bf16: halves the 1.15MB load on the startup critical
                # path; the f32 mix/demod math sees bf16-rounded weights,
                # identical to what lands in WA1/WA2 anyway)
                P = prep.tile([C, NK * 288], BF16)
                nc.sync.dma_start(
                    P[:, :], wbank.ap().rearrange("n o i ky kx -> o n (i ky kx)")
                )

                # mix[o, i*9+tap] = sum_n attn[n] * P[o, n, ...]
                # (DVE mul + ACT scaled-copy pairs, tree-added on DVE; no
                # gpsimd — each Q7 launch costs ~10us of real latency)
                mix = prep.tile([C, 288], F32, tag="mix")
                t0 = prep.tile([C, 288], F32, tag="t0")
                t1 = prep.tile([C, 288], F32, tag="t1")
                t2 = prep.tile([C, 288], F32, tag="t2")
                nc.vector.tensor_scalar_mul(t0[:, :], P[:, 0:288], attnB[:, 0:1])
                nc.scalar.activation(
                    t1[:, :], P[:, 288:576],
                    mybir.ActivationFunctionType.Copy, scale=attnB[:, 1:2])
                nc.vector.tensor_scalar_mul(
                    t2[:, :], P[:, 576:864], attnB[:, 2:3])
                nc.scalar.activation(
                    mix[:, :], P[:, 864:1152],
                    mybir.ActivationFunctionType.Copy, scale=attnB[:, 3:4])
                nc.vector.tensor_add(t0[:, :], t0[:, :], t1[:, :])
                nc.vector.tensor_add(t2[:, :], t2[:, :], mix[:, :])
                nc.vector.tensor_add(mix[:, :], t0[:, :], t2[:, :])

                # mvec[i, 1] = mod + 1 ;  m2 = mvec^2 ; m1B = mvec tiled x4
                mv = prep.tile([C, 1], F32, tag="mv")
                nc.sync.dma_start(mv[:, :], mod.ap().rearrange("a i -> i a"))
                m1 = prep.tile([C, 1], F32, tag="m1")
                nc.scalar.add(m1[:, :], mv[:, :], 1.0)
                m2 = prep.tile([C, 1], F32, tag="m2")
                nc.vector.tensor_mul(m2[:, :], m1[:, :], m1[:, :])
                for g in range(4):
                    nc.sync.dma_start(
                        m1B[32 * g:32 * g + 32, :], mod.ap().rearrange("a i -> i a")
                    )
                nc.scalar.add(m1B[:, :], m1B[:, :], 1.0)

                # demodulation: inv[o] = rsqrt(sum_{i,tap} (mix * m1[i])^2)
                sq = prep.tile([C, 288], F32, tag="tmp")
                nc.vector.tensor_mul(sq[:, :], mix[:, :], mix[:, :])
                s_oi = prep.tile([C, C], F32, tag="soi")
                nc.vector.reduce_sum(
                    s_oi[:, :],
                    sq[:, :].rearrange("p (i t) -> p i t", t=9),
                    axis=mybir.AxisListType.X,
                )
                ps_a = prep_ps.tile([C, C], F32, tag="psa")
                nc.tensor.transpose(ps_a[:, :], s_oi[:, :], id32[:, :])
                sT = prep.tile([C, C], F32, tag="soi")
                nc.vector.tensor_copy(sT[:, :], ps_a[:, :])

                ps_n = prep_ps.tile([1, C], F32, tag="psa")
                nc.tensor.matmul(
                    ps_n[:, :], m2[:, :], sT[:, :], start=True, stop=True
                )
                ns = prep.tile([1, C], F32, tag="ns")
                nc.vector.tensor_scalar_max(ns[:, :], ps_n[:, :], EPS)
                sqn = prep.tile([1, C], F32, tag="sqn")
                nc.scalar.sqrt(sqn[:, :], ns[:, :])
                inv = prep.tile([1, C], F32, tag="inv")
                nc.vector.reciprocal(inv[:, :], sqn[:, :])

                ps_i = prep_ps.tile([C, 1], F32, tag="psa")
                nc.tensor.transpose(ps_i[:, :], inv[:, :], ones1[:, 0:1])
                invT = prep.tile([C, 1], F32, tag="invT")
                nc.vector.tensor_copy(invT[:, :], ps_i[:, :])

                # Build gathered pre-transpose layouts in SBUF (DVE, within-
                # partition column shuffles), then transpose each out-column
                # block [32, Kblk] -> [Kblk, 32] so every matmul PSUM output
                # sits at a legal PE tile position.
                # mix4 view: [o, kx, ky, ci]
                mix4 = mix[:, :].rearrange(
                    "o (i ky kx) -> o kx ky i", ky=3, kx=3)
                # wtAB block (sg, c) at col (4*sg+c)*128:
                #   cols 32*(c+ky)+ci = mix[o, ci, ky, sg] * inv[o]
                wtAB = prep.tile([C, 12 * 128], F32, name="wtAB")
                nc.vector.memset(wtAB[:, :], 0.0)
                for sg in range(3):
                    for c in range(4):
                        nky = min(3, 4 - c)
                        base = (4 * sg + c) * 128 + 32 * c
                        nc.vector.tensor_scalar_mul(
                            wtAB[:, base:base + 32 * nky].rearrange(
                                "o (ky i) -> o ky i", i=C),
                            mix4[:, sg, 0:nky, :],
                            invT[:, 0:1],
                        )
                # wtAB2 block (sg, c in {2,3}) at col 128*sg + 64*(c-2):
                #   cols 32*p'+ci = w[ky=p'+4-c] (c=2: p'=0 ky=2;
                #   c=3: p'=ky-1, ky in {1,2})
                wtAB2 = prep.tile([C, 3 * 128], F32, name="wtAB2")
                nc.vector.memset(wtAB2[:, :], 0.0)
                for sg in range(3):
                    nc.vector.tensor_scalar_mul(
                        wtAB2[:, 128 * sg:128 * sg + 32],
                        mix4[:, sg, 2, :],
                        invT[:, 0:1],
                    )
                    nc.vector.tensor_scalar_mul(
                        wtAB2[:, 128 * sg + 64:128 * sg + 128].rearrange(
                            "o (ky i) -> o ky i", i=C),
                        mix4[:, sg, 1:3, :],
                        invT[:, 0:1],
                    )
                # PE transposes: wide blocks -> PSUM at partition 0
                pw1 = prep_ps.tile([128, 3 * 128], F32, tag="pw1")
                tps = [(sg, c) for sg in range(3) for c in range(4)]
                for i, (sg, c) in enumerate(tps):
                    nc.tensor.matmul(
                        pw1[:, 128 * sg + 32 * c:128 * sg + 32 * c + 32],
                        wtAB[:, (4 * sg + c) * 128:(4 * sg + c) * 128 + 128],
                        id32[:, :],
                        is_transpose=True,
                        start=(i == 0), stop=(i == len(tps) - 1),
                    )
                pw2 = prep_ps.tile([64, 3 * 128], F32, tag="pw2")
                tps2 = [(sg, c) for sg in range(3) for c in (2, 3)]
                for i, (sg, c) in enumerate(tps2):
                    nc.tensor.matmul(
                        pw2[:, 128 * sg + 32 * c:128 * sg + 32 * c + 32],
                        wtAB2[:, 128 * sg + 64 * (c - 2):
                              128 * sg + 64 * (c - 2) + 64],
                        id32[:, :],
                        is_transpose=True,
                        start=(i == 0), stop=(i == len(tps2) - 1),
                    )
                # evacuate to bf16 weight tiles, scaled by (1+mod[ci]):
                # ONE wide DVE op for WA1 (the 3 sg blocks are contiguous)
                # and ONE strided ACT op for WA2 — shortens the serial
                # evac chain the first conv Ldweights waits on.
                nc.vector.tensor_scalar_mul(
                    WA1[:, :], pw1[:, :], m1B[:, 0:1])
                nc.scalar.activation(
                    WA2[0:64, :].rearrange(
                        "p (sg q) -> p sg q", q=128)[:, :, 64:128],
                    pw2[0:64, :].rearrange(
                        "p (sg q) -> p sg q", q=128)[:, :, 64:128],
                    mybir.ActivationFunctionType.Copy,
                    scale=m1B[0:64, 0:1],
                )

            # ---------------- main conv loop ----------------
            with (
                tc.tile_pool(name="cps", bufs=8, space="PSUM") as cps,
                tc.tile_pool(name="opool", bufs=6) as opool,
            ):
                def load_slots(b, lo, nsl):
                    # DMA block b slots [lo, lo+nsl) straight into the tape
                    # (one 128-partition call; per-partition src contiguous)
                    p0 = (b * SBLK + lo) % nslot
                    s0 = b * SBLK + lo
                    nc.sync.dma_start(
                        xv[:, p0:p0 + nsl, 2:2 + W],
                        fm4[:, s0:s0 + nsl, :],
                    )

                def mm_bank(pts_t, t, sigma, which, first, last):
                    # one stream for bank t (out rows 4t+1..4t+4)
                    if which == 1:          # A1: K=128 over slot t
                        st = t % nslot
                        lhsT = WA1[:, 128 * sigma:128 * sigma + 128]
                        rhs = X[0:128, st * Wp + 1 + sigma:
                                st * Wp + 1 + sigma + W]
                        o = pts_t[:, :]
                    else:                   # A2: K=64 over slot t+1 strips 0-1
                        s1 = (t + 1) % nslot
                        lhsT = WA2[0:64, 128 * sigma:128 * sigma + 128]
                        rhs = X[0:64, s1 * Wp + 1 + sigma:
                                s1 * Wp + 1 + sigma + W]
                        o = pts_t[:, :]
                    nc.tensor.matmul(
                        o, lhsT, rhs, start=first, stop=last,
                        skip_group_check=True,
                    )

                for _rep in range(repeat):
                    # stream the whole image in, in arrival order; the first
                    # chunks are small so bank group 0 can start early
                    load_slots(0, 0, 4)
                    load_slots(0, 4, 4)
                    if SBLK > 8:
                        load_slots(0, 8, SBLK - 8)
                    for b in range(1, NBLK):
                        load_slots(b, 0, SBLK)

                    otiles = {}
                    OB = 8                  # banks per output half-block
                    NG = T // 4             # 32 groups of 4 banks
                    for k in range(NG):
                        banks = [t for t in range(4 * k - 1, 4 * k + 3)
                                 if t < T - 1]
                        if k % 2 == 0:
                            h = k // 2
                            otiles[h] = opool.tile(
                                [128, OB * W], BF16, name=f"ot{h}", tag="ot")

                        if "mm" not in skip:
                            pts = {t: cps.tile([128, W], F32,
                                               name=f"pt{t}", tag="pt")
                                   for t in banks}
                            # order: A1s0 (start, full bank) -> A2s (interior
                            # accumulates) -> A1s1 -> A1s2 (stop, full bank).
                            # The closing matmul must span all 128 partitions
                            # so every started psum group is closed.
                            for t in banks:
                                if t >= 0:
                                    mm_bank(pts[t], t, 0, 1,
                                            first=True, last=False)
                            for sigma in range(3):
                                for t in banks:
                                    mm_bank(pts[t], t, sigma, 2,
                                            first=(t < 0 and sigma == 0),
                                            last=(t < 0 and sigma == 2))
                            for sigma in (1, 2):
                                for t in banks:
                                    if t >= 0:
                                        mm_bank(pts[t], t, sigma, 1,
                                                first=False,
                                                last=(sigma == 2))

                        # evacuations (ACT) + per-block output DMA (ACT HWDGE)
                        for t in banks:
                            if "evac" in skip:
                                continue
                            if t < 0:
                                nc.scalar.copy(e0[:, :], pts[t][96:128, :])
                                if "odma" not in skip:
                                    nc.gpsimd.dma_start(
                                        osink.ap()[3, :, T - 1, :],
                                        e0[:, :],
                                    )
                            else:
                                hw_ = (t % OB) * W
                                nc.scalar.copy(
                                    otiles[t // OB][:, hw_:hw_ + W],
                                    pts[t][:, :])
                        if k % 2 == 0 and k > 0 and "odma" not in skip \
                                and "evac" not in skip:
                            hdone = k // 2 - 1
                            _emit_half_dma(nc, osink, otiles[hdone], hdone,
                                           0, OB, W)
                            otiles.pop(hdone, None)
                        if k == NG - 2 and "mm" not in skip:
                            # tail bank T-1 (rows H-3..H-1) early: its slot
                            # (T-1) loaded long ago and nothing depends on it
                            ptl = cps.tile([128, W], F32, name="ptl", tag="pt")
                            for sigma in range(3):
                                mm_bank(ptl, T - 1, sigma, 1,
                                        first=(sigma == 0), last=(sigma == 2))
                            if "evac" not in skip:
                                nc.scalar.copy(e1[:, :], ptl[0:96, :])
                                if "odma" not in skip:
                                    nc.scalar.dma_start(
                                        osink.ap().rearrange(
                                            "c o u x -> (c o) u x")[
                                            0:96, T - 1, :],
                                        e1[:, :],
                                    )
                        if k == NG - 1 and "odma" not in skip \
                                and "evac" not in skip:
                            # first chunk of the last half-block (banks
                            # 120..123 are evacuated by group NG-2)
                            _emit_half_dma(nc, osink, otiles[T // OB - 1],
                                           T // OB - 1, 0, 4, W)

                    # tail: last 3 banks stream out one by one as evacuated
                    if "evac" not in skip and "odma" not in skip:
                        hlast = T // OB - 1
                        for j in range(4, OB - 1):
                            u0 = OB * hlast + j
                            nc.scalar.dma_start(
                                osink.ap().rearrange(
                                    "c o u x -> (c o) (u x)")[
                                    :, u0 * W:(u0 + 1) * W],
                                otiles[hlast][:, j * W:(j + 1) * W],
                            )
                        otiles.pop(hlast, None)

                if timing:
                    nc.sync.dma_start(s4[:, :], osink.ap()[0, 0:1, 0, 0:NK])
                    nc.sync.dma_start(out.ap(), s4[:, :])

    nc.compile()
    return nc


OB8 = 8


def _emit_half_dma(nc, osink, ot, h, lo, nslots, W):
    """Output DMA for half-block h, banks u = 8h+lo .. 8h+lo+nslots-1.

    One 128-partition SWDGE call on the Pool queue (separate semaphore
    pool from the HWDGE input loads, so input prefetch never gates on
    output completion).  osink is strip-major [4, C, T, W] bf16; per
    (strip, channel) partition the dst run over (u, x) is contiguous.
    """
    u0 = OB8 * h + lo
    ov = osink.ap().rearrange("c o u x -> (c o) (u x)")
    nc.gpsimd.dma_start(
        ov[:, u0 * W:(u0 + nslots) * W],
        ot[:, lo * W:(lo + nslots) * W],
    )


_CACHE = {}


def _get_graph(H, W):
    key = (H, W)
    if key not in _CACHE:
        _CACHE[key] = build_graph(H, W)
    return _CACHE[key]


def _bf16():
    import ml_dtypes
    return ml_dtypes.bfloat16


def prep_weights(w):
    """weight bank -> contiguous bf16 (matches on-chip WA precision)."""
    return np.ascontiguousarray(
        np.asarray(w, dtype=np.float32).astype(_bf16()))


def restripe_fmap(fm):
    """[C, H, W] f32 -> [4, C, H//4, W] bf16 with out[g, c, t] = fm[c, 4t+g].

    The bf16 cast (round-to-nearest-even) matches the on-chip DVE convert
    it replaces, so kernel numerics are unchanged.
    """
    C_, H_, W_ = fm.shape
    return np.ascontiguousarray(
        np.asarray(fm, dtype=np.float32).astype(_bf16())
        .reshape(C_, H_ // 4, 4, W_).transpose(2, 0, 1, 3)
    )


def unstripe_out(o5, H):
    """[4, C, T, W] bf16 strip-major kernel output -> [C, H, W] f32."""
    _, C_, T_, W_ = o5.shape
    out = np.empty((C_, H, W_), np.float32)
    out[:, 1::4] = o5[0]
    out[:, 2::4] = o5[1]
    out[:, 3::4] = o5[2]
    out[:, 4::4] = o5[3][:, :T_ - 1]
    out[:, 0] = o5[3][:, T_ - 1]
    return out


def kernel(fmap, mod, kernel_mod, weights):
    B, Ci, H, Wd = fmap.shape
    nc = _get_graph(H, Wd)
    in_maps = [
        {
            "fmap": restripe_fmap(fmap[b]),
            "mod": np.ascontiguousarray(mod[b:b + 1], dtype=np.float32),
            "kernel_mod": np.ascontiguousarray(kernel_mod[b:b + 1], dtype=np.float32),
            "weights": prep_weights(weights),
        }
        for b in range(B)
    ]
    res = run_bass_kernel_spmd(nc, in_maps, core_ids=list(range(B)))
    return np.stack(
        [unstripe_out(res.results[b]["out"], H) for b in range(B)], axis=0
    )
